# revision 9
# baseline (speedup 1.0000x reference)
"""Distributed Trainium2 Bass kernel for nn_Attention_27659589386447.

Reference computation (B=2, S=2048, D=1024, H=16, HD=64):
    xq = x @ Wq.T ; xk = x @ Wq.T (key uses query weights!) ; xv = x @ Wv.T
    q = rope(xq), k = rope(xk)  -> k == q
    out = causal_softmax(q @ k.T / sqrt(HD)) @ v     per (batch, head)

Sharding (8 cores): core c -> batch b = c // 4, head group g = c % 4
(heads 4g..4g+3, feature slice m = 256*g .. 256*(g+1)).
Each core's output slice is fully independent -> no collectives.

Device algorithm per core (all matmuls bf16, accumulation f32):
  - inputs arrive host-pre-transposed/cast: xT [1024,2048] bf16,
    WqT/WvT [1024,256] bf16, rope tables [128,2048] bf16 (see pack_* below)
  - qT = WqT.T @ xT  (per 128-row tile: 2 heads packed, RoPE'd on DVE)
  - v  = xT.T @ WvT  (natural layout, ones column appended per head)
  - scoresT[j,i] = k_j . q_i computed TRANSPOSED so exp(scoresT) tiles feed
    the PV matmul directly as the moving operand (no P transposes);
    softmax denominator = ones-column of V (out row 64); no max-subtraction
    (scores bounded, exp in f32 on ACT with fused 1/sqrt(HD) scale)
  - PV: outT[65, i] += v[j,:65].T @ exp(scoresT)[j, i]  accumulated in PSUM
  - PE-transpose outT back to natural [i, 64+1], scale by 1/denominator,
    DMA out f32.

q head-dim rows are stored PERMUTED (lo halves then hi halves) so the RoPE
rotate-half becomes two 64-partition block swaps; dot products are
order-invariant so QK is unaffected. Host permutes WqT columns and the rope
tables to match.
"""

import sys

if "/opt/trn_rl_repo" not in sys.path:
    sys.path.insert(0, "/opt/trn_rl_repo")

import numpy as np
import ml_dtypes

BF16 = ml_dtypes.bfloat16

B, S, D, H = 2, 2048, 1024, 16
HD = 64
N_CORES = 8
M = 256           # features per core (4 heads)
NK = D // 128     # 8 contraction chunks
NST = S // 128    # 16 s-tiles
NSC = S // 512    # 4 s-chunks


# --------------------------------------------------------------------------
# host-side packing
# --------------------------------------------------------------------------

_PERM = np.arange(128)  # natural layout: head h at partitions 64h..64h+64


def pack_inputs(x, Wq, Wv, cos, sin):
    """Builds the 8 per-core input maps (host-side shard + layout prep)."""
    xt_b = []
    for b in range(B):
        xt_b.append(np.ascontiguousarray(x[b].T).astype(BF16))  # [1024, 2048]

    cosT = np.ascontiguousarray(cos.T).astype(np.float32)  # [64, 2048]
    sinT = np.ascontiguousarray(sin.T).astype(np.float32)
    # signed sin: s'[d] = -sin[d] (d<32), +sin[d] (d>=32)
    sinS = np.concatenate([-sinT[:32], sinT[32:]], axis=0)  # [64, 2048]
    # per-tile permuted rope rows (same permutation for both head halves)
    d_of_p = (_PERM % 64)
    cosd = cosT[d_of_p].astype(BF16)          # [128, 2048]
    sind = sinS[d_of_p].astype(BF16)          # [128, 2048]

    in_maps = []
    for c in range(N_CORES):
        b, g = c // 4, c % 4
        mr = slice(g * M, (g + 1) * M)
        wqt = np.ascontiguousarray(Wq[mr].T).astype(BF16)  # [1024, 256]
        wvt = np.ascontiguousarray(Wv[mr].T).astype(BF16)
        in_maps.append({
            "xt": xt_b[b],
            "wqt": wqt,
            "wvt": wvt,
            "cosd": cosd,
            "sind": sind,
        })
    return in_maps


def gather_outputs(results):
    out = np.empty((B, S, D), dtype=np.float32)
    for c in range(N_CORES):
        b, g = c // 4, c % 4
        out[b, :, g * M:(g + 1) * M] = results[c]["out"]
    return out


# --------------------------------------------------------------------------
# device graph
# --------------------------------------------------------------------------

def build_graph():
    from concourse import bacc, tile, mybir
    from concourse.masks import make_upper_triangular, make_identity

    bf16 = mybir.dt.bfloat16
    f32 = mybir.dt.float32

    nc = bacc.Bacc("TRN2", target_bir_lowering=False, debug=False,
                   num_devices=N_CORES)

    xt_e = nc.dram_tensor("xt", [D, S], bf16, kind="ExternalInput")
    wqt_e = nc.dram_tensor("wqt", [D, M], bf16, kind="ExternalInput")
    wvt_e = nc.dram_tensor("wvt", [D, M], bf16, kind="ExternalInput")
    cosd_e = nc.dram_tensor("cosd", [128, S], bf16, kind="ExternalInput")
    sind_e = nc.dram_tensor("sind", [128, S], bf16, kind="ExternalInput")
    out_e = nc.dram_tensor("out", [S, M], f32, kind="ExternalOutput")

    with tile.TileContext(nc) as tc:
        with (
            tc.tile_pool(name="persist", bufs=1) as pp,
            tc.tile_pool(name="work", bufs=4) as wp,
            tc.tile_pool(name="rope", bufs=3) as rp,
            tc.tile_pool(name="psmm", bufs=4, space="PSUM") as pmm,
            tc.tile_pool(name="pspv", bufs=2, space="PSUM") as ppv,
            tc.tile_pool(name="pstb", bufs=2, space="PSUM") as ptb,
        ):
            xts = [pp.tile([128, S], bf16, tag=f"xt{k}", name=f"xt{k}") for k in range(NK)]
            wqts = [pp.tile([128, M], bf16, tag=f"wq{k}", name=f"wq{k}") for k in range(NK)]
            wvts = [pp.tile([128, M], bf16, tag=f"wv{k}", name=f"wv{k}") for k in range(NK)]
            cosd = pp.tile([128, S], bf16, tag="cosd", name="cosd")
            sind = pp.tile([128, S], bf16, tag="sind", name="sind")
            q_sb = [pp.tile([128, S], bf16, tag=f"q{mt}", name=f"q{mt}") for mt in range(2)]
            v_sb = [pp.tile([128, 4 * 65], bf16, tag=f"v{st}", name=f"v{st}")
                    for st in range(NST)]
            out_sb = [pp.tile([128, M], f32, tag=f"o{st}", name=f"o{st}")
                      for st in range(NST)]
            mask = pp.tile([128, 128], bf16, tag="mask", name="mask")
            ident = pp.tile([65, 65], bf16, tag="ident", name="ident")

            # ---- constants
            make_upper_triangular(nc, mask[:, :], val=1.0, diag=True)
            make_identity(nc, ident[:, :])

            # ---- loads
            for k in range(NK):
                nc.sync.dma_start(out=wqts[k],
                                  in_=wqt_e[128 * k:128 * (k + 1), :])
            nc.sync.dma_start(out=cosd, in_=cosd_e[:, :])
            nc.sync.dma_start(out=sind, in_=sind_e[:, :])
            for k in range(NK):
                nc.sync.dma_start(out=xts[k],
                                  in_=xt_e[128 * k:128 * (k + 1), :])
            for k in range(NK):
                nc.sync.dma_start(out=wvts[k],
                                  in_=wvt_e[128 * k:128 * (k + 1), :])

            # ---- qT projection + RoPE (per head-pair tile)
            for mt in range(2):
                psums = [pmm.tile([128, 512], f32, tag="mm", name="mm")
                         for _ in range(NSC)]
                for k in range(NK):
                    for sc in range(NSC):
                        nc.tensor.matmul(
                            psums[sc][:, :],
                            wqts[k][:, 128 * mt:128 * (mt + 1)],
                            xts[k][:, 512 * sc:512 * (sc + 1)],
                            start=(k == 0), stop=(k == NK - 1),
                        )
                qraw = rp.tile([128, S], bf16, tag="qraw", name="qraw",
                               bufs=2)
                for sc in range(NSC):
                    ssl = slice(512 * sc, 512 * (sc + 1))
                    nc.scalar.copy(out=qraw[:, ssl], in_=psums[sc][:, :])
                # rotate-half per head: swap the 32-row halves of each head
                # (engines can't shift partitions; DMA can)
                qshuf = rp.tile([128, S], bf16, tag="qshuf", name="qshuf",
                                bufs=2)
                for h in range(2):
                    p = 64 * h
                    nc.sync.dma_start(out=qshuf[p:p + 32, :],
                                      in_=qraw[p + 32:p + 64, :])
                    nc.sync.dma_start(out=qshuf[p + 32:p + 64, :],
                                      in_=qraw[p:p + 32, :])
                for sc in range(NSC):
                    ssl = slice(512 * sc, 512 * (sc + 1))
                    tmp = rp.tile([128, 512], bf16, tag="rtmp", name="rtmp")
                    nc.vector.tensor_mul(tmp[:, :], qshuf[:, ssl],
                                         sind[:, ssl])
                    tmp2 = rp.tile([128, 512], bf16, tag="rtmp2", name="rtmp2")
                    nc.vector.tensor_mul(tmp2[:, :], qraw[:, ssl], cosd[:, ssl])
                    nc.vector.tensor_add(q_sb[mt][:, ssl], tmp2[:, :],
                                         tmp[:, :])

            # ---- v projection (natural layout, ones columns)
            for st in range(NST):
                psv = pmm.tile([128, 512], f32, tag="mm", name="mm")
                for k in range(NK):
                    nc.tensor.matmul(
                        psv[:, 0:M],
                        xts[k][:, 128 * st:128 * (st + 1)],
                        wvts[k][:, :],
                        start=(k == 0), stop=(k == NK - 1),
                    )
                nc.gpsimd.memset(v_sb[st][:, :], 1.0)
                for h in range(4):
                    nc.vector.tensor_copy(
                        out=v_sb[st][:, 65 * h:65 * h + 64],
                        in_=psv[:, 64 * h:64 * (h + 1)],
                    )

            # ---- attention per head-pair (row-group packed QK)
            for hp in range(2):
                qt = q_sb[hp]
                for c in range(NSC):
                    pvA = ppv.tile([65, 512], f32, tag="pv", name="pv")
                    pvB = ppv.tile([65, 512], f32, tag="pv", name="pv")
                    njt = 4 * c + 4
                    for J in range(njt):
                        off = 0 if J <= 4 * c else 128 * (J - 4 * c)
                        n = 512 - off
                        g0 = 512 * c + off
                        jsl = slice(128 * J, 128 * (J + 1))
                        isl = slice(g0, g0 + n)
                        ptiles = []
                        for a, (p0, p1) in enumerate(((0, 64), (64, 128))):
                            psqk = pmm.tile([128, 512], f32, tag="mm", name="mm")
                            nc.tensor.matmul(
                                psqk[:, 0:n],
                                qt[p0:p1, jsl],
                                qt[p0:p1, isl],
                                start=True, stop=True,
                                tile_position=(p0, 0),
                            )
                            pt = wp.tile([128, 512], bf16, tag="pt", name="pt")
                            nc.scalar.activation(
                                out=pt[:, 0:n], in_=psqk[:, 0:n],
                                func=mybir.ActivationFunctionType.Exp,
                                scale=0.125,
                            )
                            if J >= 4 * c:
                                nc.vector.tensor_mul(pt[:, 0:128],
                                                     pt[:, 0:128],
                                                     mask[:, :])
                            ptiles.append(pt)
                        for a, pv in enumerate((pvA, pvB)):
                            h = 2 * hp + a
                            nc.tensor.matmul(
                                pv[:, off:off + n],
                                v_sb[J][:, 65 * h:65 * h + 65],
                                ptiles[a][:, 0:n],
                                start=(J == 0), stop=(J == njt - 1),
                                skip_group_check=True,
                            )
                    # ---- transpose back + normalize
                    for a, pv in enumerate((pvA, pvB)):
                        h = 2 * hp + a
                        otsb = wp.tile([65, 512], bf16, tag="otsb", name="otsb")
                        nc.vector.tensor_copy(out=otsb[:, :], in_=pv[:, :])
                        for q4 in range(4):
                            st = 4 * c + q4
                            tb = ptb.tile([128, 65], bf16, tag="tb", name="tb")
                            nc.tensor.transpose(
                                tb[:, :],
                                otsb[:, 128 * q4:128 * (q4 + 1)],
                                ident[:, :],
                            )
                            rec = wp.tile([128, 1], f32, tag="rec", name="rec")
                            nc.vector.reciprocal(out=rec[:, :],
                                                 in_=tb[:, 64:65])
                            nc.vector.tensor_scalar_mul(
                                out_sb[st][:, 64 * h:64 * (h + 1)],
                                tb[:, 0:64],
                                rec[:, :],
                            )

            # ---- store
            for st in range(NST):
                nc.sync.dma_start(out=out_e[128 * st:128 * (st + 1), :],
                                  in_=out_sb[st])

    nc.compile()
    return nc


_NC = None


def get_graph():
    global _NC
    if _NC is None:
        _NC = build_graph()
    return _NC


# --------------------------------------------------------------------------
# execution (PJRT via axon), cached jitted runner
# --------------------------------------------------------------------------

_RUNNER = None


class _Runner:
    """Builds the sharded jit once; callable with a list of 8 in_maps."""

    def __init__(self, nc):
        import jax
        import numpy as _np
        from jax.sharding import Mesh, PartitionSpec
        from jax.experimental.shard_map import shard_map
        from concourse import bass2jax, mybir
        from concourse.bass2jax import (_bass_exec_p, install_neuronx_cc_hook,
                                        partition_id_tensor)

        install_neuronx_cc_hook()
        self.jax = jax
        self.nc = nc
        partition_name = (nc.partition_id_tensor.name
                          if nc.partition_id_tensor else None)

        in_names = []
        out_names = []
        out_avals = []
        zero_shapes = []
        for alloc in nc.m.functions[0].allocations:
            if not isinstance(alloc, mybir.MemoryLocationSet):
                continue
            name = alloc.memorylocations[0].name
            if alloc.kind == "ExternalInput":
                if name != partition_name:
                    in_names.append(name)
            elif alloc.kind == "ExternalOutput":
                shape = tuple(alloc.tensor_shape)
                dtype = mybir.dt.np(alloc.dtype)
                out_names.append(name)
                out_avals.append(jax.core.ShapedArray(shape, dtype))
                zero_shapes.append((shape, dtype))
        self.in_names = list(in_names)
        self.out_names = out_names
        self.out_avals = out_avals
        self.zero_shapes = zero_shapes
        n_params = len(in_names)
        n_outs = len(out_names)
        all_in_names = in_names + out_names
        if partition_name is not None:
            all_in_names = all_in_names + [partition_name]

        def _body(*args):
            operands = list(args)
            if partition_name is not None:
                operands.append(partition_id_tensor())
            outs = _bass_exec_p.bind(
                *operands,
                out_avals=tuple(out_avals),
                in_names=tuple(all_in_names),
                out_names=tuple(out_names),
                lowering_input_output_aliases=(),
                sim_require_finite=True,
                sim_require_nnan=True,
                nc=nc,
            )
            return tuple(outs)

        devices = jax.devices()[:N_CORES]
        mesh = Mesh(np.asarray(devices), ("core",))
        in_specs = (PartitionSpec("core"),) * (n_params + n_outs)
        out_specs = (PartitionSpec("core"),) * n_outs
        donate = tuple(range(n_params, n_params + n_outs))
        self.sharded = jax.jit(
            shard_map(_body, mesh=mesh, in_specs=in_specs,
                      out_specs=out_specs, check_rep=False),
            donate_argnums=donate, keep_unused=True,
        )

    def concat_inputs(self, in_maps):
        return [
            np.concatenate([np.asarray(in_maps[c][n]) for c in range(N_CORES)],
                           axis=0)
            for n in self.in_names
        ]

    def make_zeros(self):
        return [np.zeros((N_CORES * s[0], *s[1:]), d)
                for (s, d) in self.zero_shapes]

    def __call__(self, in_maps):
        concat_in = self.concat_inputs(in_maps)
        out_arrs = self.sharded(*concat_in, *self.make_zeros())
        return [
            {name: np.asarray(out_arrs[i]).reshape(
                N_CORES, *self.out_avals[i].shape)[c]
             for i, name in enumerate(self.out_names)}
            for c in range(N_CORES)
        ]


def get_runner():
    global _RUNNER
    if _RUNNER is None:
        _RUNNER = _Runner(get_graph())
    return _RUNNER


def kernel(x, Wq, Wv, cos, sin):
    x = np.asarray(x, dtype=np.float32)
    Wq = np.asarray(Wq, dtype=np.float32)
    Wv = np.asarray(Wv, dtype=np.float32)
    cos = np.asarray(cos, dtype=np.float32)
    sin = np.asarray(sin, dtype=np.float32)
    in_maps = pack_inputs(x, Wq, Wv, cos, sin)
    results = get_runner()(in_maps)
    return gather_outputs(results)


# revision 10
# speedup vs baseline: 1.1575x; 1.1575x over previous
"""Distributed Trainium2 Bass kernel for nn_Attention_27659589386447.

Reference computation (B=2, S=2048, D=1024, H=16, HD=64):
    xq = x @ Wq.T ; xk = x @ Wq.T (key uses query weights!) ; xv = x @ Wv.T
    q = rope(xq), k = rope(xk)  -> k == q
    out = causal_softmax(q @ k.T / sqrt(HD)) @ v     per (batch, head)

Sharding (8 cores): core c -> batch b = c // 4, head group g = c % 4
(heads 4g..4g+3, feature slice m = 256*g .. 256*(g+1)).
Each core's output slice is fully independent -> no collectives.

Device algorithm per core (all matmuls bf16, accumulation f32):
  - inputs arrive host-pre-transposed/cast: xT [1024,2048] bf16,
    WqT/WvT [1024,256] bf16, rope tables [128,2048] bf16 (see pack_* below)
  - qT = WqT.T @ xT  (per 128-row tile: 2 heads packed, RoPE'd on DVE)
  - v  = xT.T @ WvT  (natural layout, ones column appended per head)
  - scoresT[j,i] = k_j . q_i computed TRANSPOSED so exp(scoresT) tiles feed
    the PV matmul directly as the moving operand (no P transposes);
    softmax denominator = ones-column of V (out row 64); no max-subtraction
    (scores bounded, exp in f32 on ACT with fused 1/sqrt(HD) scale)
  - PV: outT[65, i] += v[j,:65].T @ exp(scoresT)[j, i]  accumulated in PSUM
  - PE-transpose outT back to natural [i, 64+1], scale by 1/denominator,
    DMA out f32.

q head-dim rows are stored PERMUTED (lo halves then hi halves) so the RoPE
rotate-half becomes two 64-partition block swaps; dot products are
order-invariant so QK is unaffected. Host permutes WqT columns and the rope
tables to match.
"""

import sys

if "/opt/trn_rl_repo" not in sys.path:
    sys.path.insert(0, "/opt/trn_rl_repo")

import numpy as np
import ml_dtypes

BF16 = ml_dtypes.bfloat16

B, S, D, H = 2, 2048, 1024, 16
HD = 64
N_CORES = 8
M = 256           # features per core (4 heads)
NK = D // 128     # 8 contraction chunks
NST = S // 128    # 16 s-tiles
NSC = S // 512    # 4 s-chunks


# --------------------------------------------------------------------------
# host-side packing
# --------------------------------------------------------------------------

_PERM = np.arange(128)  # natural layout: head h at partitions 64h..64h+64


def pack_inputs(x, Wq, Wv, cos, sin):
    """Builds the 8 per-core input maps (host-side shard + layout prep)."""
    xt_b = []
    for b in range(B):
        xt_b.append(np.ascontiguousarray(x[b].T).astype(BF16))  # [1024, 2048]

    cosT = np.ascontiguousarray(cos.T).astype(np.float32)  # [64, 2048]
    sinT = np.ascontiguousarray(sin.T).astype(np.float32)
    # signed sin: s'[d] = -sin[d] (d<32), +sin[d] (d>=32)
    sinS = np.concatenate([-sinT[:32], sinT[32:]], axis=0)  # [64, 2048]
    # per-tile permuted rope rows (same permutation for both head halves)
    d_of_p = (_PERM % 64)
    cosd = cosT[d_of_p].astype(BF16)          # [128, 2048]
    sind = sinS[d_of_p].astype(BF16)          # [128, 2048]

    in_maps = []
    for c in range(N_CORES):
        b, g = c // 4, c % 4
        mr = slice(g * M, (g + 1) * M)
        wqt = np.ascontiguousarray(Wq[mr].T).astype(BF16)  # [1024, 256]
        wvt = np.ascontiguousarray(Wv[mr].T).astype(BF16)
        in_maps.append({
            "xt": xt_b[b],
            "wqt": wqt,
            "wvt": wvt,
            "cosd": cosd,
            "sind": sind,
        })
    return in_maps


def gather_outputs(results):
    out = np.empty((B, S, D), dtype=np.float32)
    for c in range(N_CORES):
        b, g = c // 4, c % 4
        out[b, :, g * M:(g + 1) * M] = results[c]["out"]
    return out


# --------------------------------------------------------------------------
# device graph
# --------------------------------------------------------------------------

def build_graph():
    from concourse import bacc, tile, mybir
    from concourse.masks import make_upper_triangular, make_identity

    bf16 = mybir.dt.bfloat16
    f32 = mybir.dt.float32

    nc = bacc.Bacc("TRN2", target_bir_lowering=False, debug=False,
                   num_devices=N_CORES)

    xt_e = nc.dram_tensor("xt", [D, S], bf16, kind="ExternalInput")
    wqt_e = nc.dram_tensor("wqt", [D, M], bf16, kind="ExternalInput")
    wvt_e = nc.dram_tensor("wvt", [D, M], bf16, kind="ExternalInput")
    cosd_e = nc.dram_tensor("cosd", [128, S], bf16, kind="ExternalInput")
    sind_e = nc.dram_tensor("sind", [128, S], bf16, kind="ExternalInput")
    out_e = nc.dram_tensor("out", [S, M], f32, kind="ExternalOutput")

    with tile.TileContext(nc) as tc:
        with (
            tc.tile_pool(name="persist", bufs=1) as pp,
            tc.tile_pool(name="work", bufs=4) as wp,
            tc.tile_pool(name="rope", bufs=3) as rp,
            tc.tile_pool(name="psmm", bufs=4, space="PSUM") as pmm,
            tc.tile_pool(name="pspv", bufs=2, space="PSUM") as ppv,
            tc.tile_pool(name="pstb", bufs=2, space="PSUM") as ptb,
        ):
            xts = [pp.tile([128, S], bf16, tag=f"xt{k}", name=f"xt{k}") for k in range(NK)]
            wqts = [pp.tile([128, M], bf16, tag=f"wq{k}", name=f"wq{k}") for k in range(NK)]
            wvts = [pp.tile([128, M], bf16, tag=f"wv{k}", name=f"wv{k}") for k in range(NK)]
            cosd = pp.tile([128, S], bf16, tag="cosd", name="cosd")
            sind = pp.tile([128, S], bf16, tag="sind", name="sind")
            q_sb = [pp.tile([128, S], bf16, tag=f"q{mt}", name=f"q{mt}") for mt in range(2)]
            v_sb = [pp.tile([128, 4 * 65], bf16, tag=f"v{st}", name=f"v{st}")
                    for st in range(NST)]
            out_sb = [pp.tile([128, M], f32, tag=f"o{st}", name=f"o{st}")
                      for st in range(NST)]
            mask = pp.tile([128, 128], bf16, tag="mask", name="mask")
            ident = pp.tile([65, 65], bf16, tag="ident", name="ident")

            # ---- constants
            make_upper_triangular(nc, mask[:, :], val=1.0, diag=True)
            make_identity(nc, ident[:, :])

            # ---- loads
            for k in range(NK):
                nc.sync.dma_start(out=wqts[k],
                                  in_=wqt_e[128 * k:128 * (k + 1), :])
            nc.sync.dma_start(out=cosd, in_=cosd_e[:, :])
            nc.sync.dma_start(out=sind, in_=sind_e[:, :])
            for k in range(NK):
                nc.sync.dma_start(out=xts[k],
                                  in_=xt_e[128 * k:128 * (k + 1), :])
            for k in range(NK):
                nc.sync.dma_start(out=wvts[k],
                                  in_=wvt_e[128 * k:128 * (k + 1), :])

            # ---- qT projection + RoPE (per head-pair tile)
            for mt in range(2):
                psums = [pmm.tile([128, 512], f32, tag="mm", name="mm")
                         for _ in range(NSC)]
                for k in range(NK):
                    for sc in range(NSC):
                        nc.tensor.matmul(
                            psums[sc][:, :],
                            wqts[k][:, 128 * mt:128 * (mt + 1)],
                            xts[k][:, 512 * sc:512 * (sc + 1)],
                            start=(k == 0), stop=(k == NK - 1),
                        )
                qraw = rp.tile([128, S], bf16, tag="qraw", name="qraw",
                               bufs=2)
                for sc in range(NSC):
                    ssl = slice(512 * sc, 512 * (sc + 1))
                    nc.scalar.copy(out=qraw[:, ssl], in_=psums[sc][:, :])
                # rotate-half per head: swap the 32-row halves of each head
                # (engines can't shift partitions; DMA can)
                qshuf = rp.tile([128, S], bf16, tag="qshuf", name="qshuf",
                                bufs=2)
                for h in range(2):
                    p = 64 * h
                    nc.sync.dma_start(out=qshuf[p:p + 32, :],
                                      in_=qraw[p + 32:p + 64, :])
                    nc.sync.dma_start(out=qshuf[p + 32:p + 64, :],
                                      in_=qraw[p:p + 32, :])
                for sc in range(NSC):
                    ssl = slice(512 * sc, 512 * (sc + 1))
                    tmp = rp.tile([128, 512], bf16, tag="rtmp", name="rtmp")
                    nc.vector.tensor_mul(tmp[:, :], qshuf[:, ssl],
                                         sind[:, ssl])
                    tmp2 = rp.tile([128, 512], bf16, tag="rtmp2", name="rtmp2")
                    nc.vector.tensor_mul(tmp2[:, :], qraw[:, ssl], cosd[:, ssl])
                    nc.vector.tensor_add(q_sb[mt][:, ssl], tmp2[:, :],
                                         tmp[:, :])

            # ---- v projection (natural layout, ones columns)
            for st in range(NST):
                psv = pmm.tile([128, 512], f32, tag="mm", name="mm")
                for k in range(NK):
                    nc.tensor.matmul(
                        psv[:, 0:M],
                        xts[k][:, 128 * st:128 * (st + 1)],
                        wvts[k][:, :],
                        start=(k == 0), stop=(k == NK - 1),
                    )
                nc.gpsimd.memset(v_sb[st][:, :], 1.0)
                for h in range(4):
                    nc.vector.tensor_copy(
                        out=v_sb[st][:, 65 * h:65 * h + 64],
                        in_=psv[:, 64 * h:64 * (h + 1)],
                    )

            # ---- attention per head-pair (row-group packed QK)
            for hp in range(2):
                qt = q_sb[hp]
                for c in range(NSC):
                    pvA = ppv.tile([65, 512], f32, tag="pv", name="pv")
                    pvB = ppv.tile([65, 512], f32, tag="pv", name="pv")
                    njt = 4 * c + 4
                    for J in range(njt):
                        off = 0 if J <= 4 * c else 128 * (J - 4 * c)
                        n = 512 - off
                        g0 = 512 * c + off
                        jsl = slice(128 * J, 128 * (J + 1))
                        isl = slice(g0, g0 + n)
                        ptiles = []
                        for a, (p0, p1) in enumerate(((0, 64), (64, 128))):
                            psqk = pmm.tile([128, 512], f32, tag="mm", name="mm")
                            nc.tensor.matmul(
                                psqk[:, 0:n],
                                qt[p0:p1, jsl],
                                qt[p0:p1, isl],
                                start=True, stop=True,
                                tile_position=(p0, 0),
                            )
                            pt = wp.tile([128, 512], bf16, tag="pt", name="pt")
                            nc.scalar.activation(
                                out=pt[:, 0:n], in_=psqk[:, 0:n],
                                func=mybir.ActivationFunctionType.Exp,
                                scale=0.125,
                            )
                            if J >= 4 * c:
                                nc.vector.tensor_mul(pt[:, 0:128],
                                                     pt[:, 0:128],
                                                     mask[:, :])
                            ptiles.append(pt)
                        for a, pv in enumerate((pvA, pvB)):
                            h = 2 * hp + a
                            nc.tensor.matmul(
                                pv[:, off:off + n],
                                v_sb[J][:, 65 * h:65 * h + 65],
                                ptiles[a][:, 0:n],
                                start=(J == 0), stop=(J == njt - 1),
                                skip_group_check=True,
                            )
                    # ---- transpose back + normalize
                    for a, pv in enumerate((pvA, pvB)):
                        h = 2 * hp + a
                        otsb = wp.tile([65, 512], bf16, tag="otsb", name="otsb")
                        nc.vector.tensor_copy(out=otsb[:, :], in_=pv[:, :])
                        for q4 in range(4):
                            st = 4 * c + q4
                            tb = ptb.tile([128, 65], bf16, tag="tb", name="tb")
                            nc.tensor.transpose(
                                tb[:, :],
                                otsb[:, 128 * q4:128 * (q4 + 1)],
                                ident[:, :],
                            )
                            rec = wp.tile([128, 1], f32, tag="rec", name="rec")
                            nc.vector.reciprocal(out=rec[:, :],
                                                 in_=tb[:, 64:65])
                            nc.vector.tensor_scalar_mul(
                                out_sb[st][:, 64 * h:64 * (h + 1)],
                                tb[:, 0:64],
                                rec[:, :],
                            )

            # ---- store
            for st in range(NST):
                nc.sync.dma_start(out=out_e[128 * st:128 * (st + 1), :],
                                  in_=out_sb[st])

    nc.compile()
    return nc


_NC = None


def get_graph():
    global _NC
    if _NC is None:
        _NC = build_graph()
    return _NC


# --------------------------------------------------------------------------
# execution (PJRT via axon), cached jitted runner
# --------------------------------------------------------------------------

_RUNNER = None


class _Runner:
    """Builds the sharded jit once; callable with a list of 8 in_maps."""

    def __init__(self, nc):
        import jax
        import numpy as _np
        from jax.sharding import Mesh, PartitionSpec
        from jax.experimental.shard_map import shard_map
        from concourse import bass2jax, mybir
        from concourse.bass2jax import (_bass_exec_p, install_neuronx_cc_hook,
                                        partition_id_tensor)

        install_neuronx_cc_hook()
        self.jax = jax
        self.nc = nc
        partition_name = (nc.partition_id_tensor.name
                          if nc.partition_id_tensor else None)

        in_names = []
        out_names = []
        out_avals = []
        zero_shapes = []
        for alloc in nc.m.functions[0].allocations:
            if not isinstance(alloc, mybir.MemoryLocationSet):
                continue
            name = alloc.memorylocations[0].name
            if alloc.kind == "ExternalInput":
                if name != partition_name:
                    in_names.append(name)
            elif alloc.kind == "ExternalOutput":
                shape = tuple(alloc.tensor_shape)
                dtype = mybir.dt.np(alloc.dtype)
                out_names.append(name)
                out_avals.append(jax.core.ShapedArray(shape, dtype))
                zero_shapes.append((shape, dtype))
        self.in_names = list(in_names)
        self.out_names = out_names
        self.out_avals = out_avals
        self.zero_shapes = zero_shapes
        n_params = len(in_names)
        n_outs = len(out_names)
        all_in_names = in_names + out_names
        if partition_name is not None:
            all_in_names = all_in_names + [partition_name]

        def _body(*args):
            operands = list(args)
            if partition_name is not None:
                operands.append(partition_id_tensor())
            outs = _bass_exec_p.bind(
                *operands,
                out_avals=tuple(out_avals),
                in_names=tuple(all_in_names),
                out_names=tuple(out_names),
                lowering_input_output_aliases=(),
                sim_require_finite=True,
                sim_require_nnan=True,
                nc=nc,
            )
            return tuple(outs)

        devices = jax.devices()[:N_CORES]
        mesh = Mesh(np.asarray(devices), ("core",))
        self.mesh = mesh
        in_specs = (PartitionSpec("core"),) * (n_params + n_outs)
        out_specs = (PartitionSpec("core"),) * n_outs
        donate = tuple(range(n_params, n_params + n_outs))
        self.sharded = jax.jit(
            shard_map(_body, mesh=mesh, in_specs=in_specs,
                      out_specs=out_specs, check_rep=False),
            donate_argnums=donate, keep_unused=True,
        )

    def concat_inputs(self, in_maps):
        return [
            np.concatenate([np.asarray(in_maps[c][n]) for c in range(N_CORES)],
                           axis=0)
            for n in self.in_names
        ]

    def make_zeros(self):
        return [np.zeros((N_CORES * s[0], *s[1:]), d)
                for (s, d) in self.zero_shapes]

    def __call__(self, in_maps):
        concat_in = self.concat_inputs(in_maps)
        out_arrs = self.sharded(*concat_in, *self.make_zeros())
        return [
            {name: np.asarray(out_arrs[i]).reshape(
                N_CORES, *self.out_avals[i].shape)[c]
             for i, name in enumerate(self.out_names)}
            for c in range(N_CORES)
        ]


def get_runner():
    global _RUNNER
    if _RUNNER is None:
        _RUNNER = _Runner(get_graph())
    return _RUNNER


def kernel(x, Wq, Wv, cos, sin):
    x = np.asarray(x, dtype=np.float32)
    Wq = np.asarray(Wq, dtype=np.float32)
    Wv = np.asarray(Wv, dtype=np.float32)
    cos = np.asarray(cos, dtype=np.float32)
    sin = np.asarray(sin, dtype=np.float32)
    in_maps = pack_inputs(x, Wq, Wv, cos, sin)
    results = get_runner()(in_maps)
    return gather_outputs(results)
